# revision 1
# baseline (speedup 1.0000x reference)
"""Expert-parallel MoE MLP kernel for Trainium2 (8 NeuronCores).

Problem: x[B=2,S=1024,H=1024] f32, expert_indices[B,S] int, 16 experts,
gate/up_proj[E,H,I], down_proj[E,I,H] (H=I=1024):
    out[n] = silu(x_n @ Wg[e_n]) * (x_n @ Wu[e_n]) @ Wd[e_n].T

Sharding: expert parallelism — core c owns experts {2c, 2c+1}. The host
groups tokens by expert (the "all-to-all dispatch" runs on host since the
kernel contract is full-input -> full-output), pads each expert's token
block to a fixed capacity, and each core runs dense per-expert GEMMs.

Device layout (per core, per expert e) keeps features on partitions so no
on-chip transposes are needed:
    xt    = X_e^T                [H=1024, P]
    Gt[i,n] = sum_h Wg[h,i]*xt[h,n];  inter = silu(Gt)*Ut
    Out^T[j,n] = sum_k WdT[k,j]*inter[k,n]   (WdT = Wd.T, host-transposed)

Matmuls run as float32r (TF32-like, 10 explicit HW mantissa bits,
~1.5e-4 rel error) at 1 cycle/row for moving dim >=256; all operands are
pre-rounded (unrounded f32r operands hard-fault the exec unit).

Perf structure:
  - weights are 24 MB/core of mandatory HBM traffic (the roofline); they
    are host-packed partition-major so each 1 MB DMA chunk moves 8 KB
    contiguous per-partition runs (near line-rate)
  - expert-0 gate weights + xt are interleaved per-chunk at the head of
    the HWDGE FIFO, and phases accumulate h-outer into 8 PSUM banks, so
    the first matmul starts after ~1 MB instead of ~6 MB
  - token blocks are DMA'd at their real (padded-to-16) width; the matmul
    still streams 256 columns, with the pad region zeroed once on-chip
  - each expert's outputs are staged in SBUF and shipped as one DMA so
    the kernel tail isn't serialized on 8 small-DMA dispatches
"""

import math

import numpy as np

E = 16
H = 1024
HT = 8          # H / 128 partition tiles
HB = 2          # h-blocks per weight DMA chunk (1 MB chunks)
N_CORES = 8
EPC = E // N_CORES  # experts per core
CW = 256        # matmul moving-dim width (>=256 keeps f32r at 1 cyc/row)

_NC_CACHE = {}


def _round_f32r(a, mant=16):
    """Round-to-nearest to `mant` explicit mantissa bits (f32r operand prep)."""
    xi = np.ascontiguousarray(a, dtype=np.float32).view(np.uint32).astype(np.uint64)
    drop = 23 - mant
    half = np.uint64(1 << (drop - 1))
    mask = np.uint64((~((1 << drop) - 1)) & 0xFFFFFFFF)
    return ((xi + half) & mask).astype(np.uint32).view(np.float32)


def _build_nc(ch: int, pio: int, mmdt: str = "float32r"):
    """One SPMD program: EPC experts, ch chunks of CW token-slots per expert,
    pio real (DMA'd) token columns per expert, pio <= ch*CW."""
    import concourse.tile as tile
    from concourse import bacc, mybir
    from concourse.bass import ts

    f32 = mybir.dt.float32
    f32r = mybir.dt[mmdt]
    P = ch * CW
    NHB = HT // HB

    nc = bacc.Bacc("TRN2", target_bir_lowering=False, debug=False,
                   num_devices=N_CORES)
    # weights packed partition-major: w[e, proj, p, h, :] = Wproj[e][h*128+p, :]
    w = nc.dram_tensor("w", [EPC, 3, 128, HT, H], f32r, kind="ExternalInput")
    xt = nc.dram_tensor("xt", [EPC, 128, HT, pio], f32r, kind="ExternalInput")
    out = nc.dram_tensor("out", [EPC, 128, HT, pio], f32, kind="ExternalOutput")

    assert ch == 1, "token groups above one chunk go through the round loop"
    with tile.TileContext(nc) as tc:
        with (
            tc.tile_pool(name="wp", bufs=16) as wp,
            tc.tile_pool(name="xp", bufs=1) as xp,
            tc.tile_pool(name="gp", bufs=2) as gp,
            tc.tile_pool(name="ip", bufs=2) as ip,
            tc.tile_pool(name="op", bufs=2) as op,
            tc.tile_pool(name="ps", bufs=8, space="PSUM") as ps,
        ):
            x_sb = xp.tile([128, EPC, HT, P], f32r)
            if pio < P:
                pad = x_sb[:, :, :, pio:P]
                nc.vector.memset(pad.bitcast(f32) if mmdt == "float32r" else pad,
                                 0.0)

            # expert-0 gate weights and xt interleaved per-chunk at the head
            # of the HWDGE FIFO: the first matmul needs only chunk 0 of each.
            NS = 4                  # weight tile = quarter projection (1 MB)
            HH = HT // NS           # h-rows per weight tile
            halves = {}

            def w_half(e, proj, hf):
                t = wp.tile([128, HH, H], f32r, tag="w",
                            name=f"wh{e}{proj}{hf}")
                nc.sync.dma_start(t[:], w[e, proj, :, ts(hf, HH), :])
                return t

            # expert-0 gate weights and xt interleaved at the FIFO head
            for hf in range(NS):
                if hf % 2 == 0:
                    nc.sync.dma_start(
                        x_sb[:, 0, ts(hf // 2, HT // 2), 0:pio],
                        xt[0, :, ts(hf // 2, HT // 2), :])
                halves[0, 0, hf] = w_half(0, 0, hf)

            for e in range(EPC):
                for proj in range(3):
                    for hf in range(NS):
                        if (e, proj, hf) not in halves:
                            halves[e, proj, hf] = w_half(e, proj, hf)
                if e + 1 < EPC:
                    nc.sync.dma_start(x_sb[:, e + 1, :, 0:pio], xt[e + 1])

                def wsl(proj, h, col):
                    return halves[e, proj, h // HH][:, h % HH, col]

                g_sb = gp.tile([128, HT, P], f32)       # silu(Gt)
                i_sb = ip.tile([128, HT, P], f32r)      # inter = silu(Gt)*Ut
                o_sb = op.tile([128, HT, pio], f32, tag="o")
                for c in range(ch):
                    cs = c * CW
                    # gate: h-outer accumulation into 8 PSUM banks
                    g_ps = [ps.tile([128, CW], f32, tag="ps", name=f"gps{i_}")
                            for i_ in range(HT)]
                    for h in range(HT):
                        for i in range(HT):
                            nc.tensor.matmul(
                                g_ps[i][:], wsl(0, h, ts(i, 128)),
                                x_sb[:, e, h, cs:cs + CW],
                                start=(h == 0), stop=(h == HT - 1))
                    for i in range(HT):
                        nc.scalar.activation(
                            g_sb[:, i, cs:cs + CW], g_ps[i][:],
                            mybir.ActivationFunctionType.Silu)
                    # up
                    u_ps = [ps.tile([128, CW], f32, tag="ps", name=f"ups{i_}")
                            for i_ in range(HT)]
                    for h in range(HT):
                        for i in range(HT):
                            nc.tensor.matmul(
                                u_ps[i][:], wsl(1, h, ts(i, 128)),
                                x_sb[:, e, h, cs:cs + CW],
                                start=(h == 0), stop=(h == HT - 1))
                    for i in range(HT):
                        nc.vector.tensor_mul(
                            i_sb[:, i, cs:cs + CW],
                            g_sb[:, i, cs:cs + CW], u_ps[i][:])
                    # down
                    o_ps = [ps.tile([128, CW], f32, tag="ps", name=f"ops{i_}")
                            for i_ in range(HT)]
                    for k in range(HT):
                        for j in range(HT):
                            nc.tensor.matmul(
                                o_ps[j][:], wsl(2, k, ts(j, 128)),
                                i_sb[:, k, cs:cs + CW],
                                start=(k == 0), stop=(k == HT - 1))
                    lo, hi = cs, min(cs + CW, pio)
                    for j in range(HT):
                        if hi > lo:
                            nc.vector.tensor_copy(
                                o_sb[:, j, lo:hi], o_ps[j][:, 0:hi - lo])
                # staged outputs, two half-DMAs so the first overlaps the
                # second half's down matmuls
                nc.sync.dma_start(out[e, :, 0:HT // 2, :], o_sb[:, 0:HT // 2, :])
                nc.sync.dma_start(out[e, :, HT // 2:HT, :], o_sb[:, HT // 2:HT, :])
    nc.compile()
    return nc


MM_DTYPE = "float32r"     # "float32r" (TF32, ~2.4e-4) or "bfloat16" (~2e-3, 2x DMA win)


def _get_nc(ch: int, pio: int):
    key = (ch, pio, MM_DTYPE)
    if key not in _NC_CACHE:
        _NC_CACHE[key] = _build_nc(ch, pio, MM_DTYPE)
    return _NC_CACHE[key]


_ROUND_CAP = 256          # max tokens/expert per device round (one chunk)


def _kernel_once(x, expert_indices, gate_proj, up_proj, down_proj):
    from concourse.bass_utils import run_bass_kernel_spmd

    x = np.ascontiguousarray(x, dtype=np.float32)
    gate_proj = np.ascontiguousarray(gate_proj, dtype=np.float32)
    up_proj = np.ascontiguousarray(up_proj, dtype=np.float32)
    down_proj = np.ascontiguousarray(down_proj, dtype=np.float32)
    b, s, h = x.shape
    assert (h, gate_proj.shape) == (H, (E, H, H)), (x.shape, gate_proj.shape)

    n = b * s
    xf = x.reshape(n, h)
    idx = np.asarray(expert_indices).reshape(n).astype(np.int64)

    order = np.argsort(idx, kind="stable")       # token ids grouped by expert
    counts = np.bincount(idx, minlength=E)
    starts = np.zeros(E + 1, dtype=np.int64)
    np.cumsum(counts, out=starts[1:])
    maxc = int(counts.max())
    ch = max(1, math.ceil(maxc / CW))
    pio = min(ch * CW, max(16, 16 * math.ceil(maxc / 16)))

    # per-core inputs; weights packed partition-major [EPC,3,128,HT,H]
    if MM_DTYPE == "bfloat16":
        import ml_dtypes
        def _prep(a):
            return np.ascontiguousarray(a, dtype=np.float32).astype(
                np.dtype(ml_dtypes.bfloat16))
    else:
        _prep = _round_f32r
    wr = _prep(
        np.stack([gate_proj, up_proj, down_proj.transpose(0, 2, 1)], axis=1)
    ).reshape(N_CORES, EPC, 3, HT, 128, H).transpose(0, 1, 2, 4, 3, 5)
    in_maps = []
    tok_ids = []
    for c in range(N_CORES):
        xt_c = np.zeros((EPC, H, pio), dtype=np.float32)
        toks = []
        for le in range(EPC):
            e = c * EPC + le
            te = order[starts[e]:starts[e + 1]]
            toks.append(te)
            xt_c[le, :, :len(te)] = xf[te].T
        tok_ids.append(toks)
        in_maps.append({
            "w": np.ascontiguousarray(wr[c]),
            "xt": _prep(xt_c).reshape(EPC, HT, 128, pio)
                  .transpose(0, 2, 1, 3).copy(),
        })

    nc = _get_nc(ch, pio)
    res = run_bass_kernel_spmd(nc, in_maps, core_ids=list(range(N_CORES)))

    out = np.empty((n, h), dtype=np.float32)
    for c in range(N_CORES):
        o = res.results[c]["out"]                # [EPC, 128, HT, pio]
        for le in range(EPC):
            te = tok_ids[c][le]
            oe = o[le].transpose(1, 0, 2).reshape(h, pio)   # [H, pio]
            out[te] = oe[:, :len(te)].T
    return out.reshape(b, s, h)


def kernel(x, expert_indices, gate_proj, up_proj, down_proj):
    """Full-input -> full-output entry point.

    Tokens-per-expert above _ROUND_CAP (pathological skew; SBUF bound)
    are handled by running the device kernel in multiple rounds over
    disjoint token slices — outputs are per-token independent."""
    idx = np.asarray(expert_indices)
    counts = np.bincount(idx.reshape(-1).astype(np.int64), minlength=E)
    if counts.max() <= _ROUND_CAP:
        return _kernel_once(x, expert_indices, gate_proj, up_proj, down_proj)

    b, s, h = x.shape
    n = b * s
    xf = np.ascontiguousarray(x, dtype=np.float32).reshape(n, h)
    idxf = idx.reshape(n).astype(np.int64)
    order = np.argsort(idxf, kind="stable")
    starts = np.zeros(E + 1, dtype=np.int64)
    np.cumsum(np.bincount(idxf, minlength=E), out=starts[1:])
    out = np.empty((n, h), dtype=np.float32)
    rounds = math.ceil(counts.max() / _ROUND_CAP)
    for r in range(rounds):
        sel = np.concatenate([
            order[starts[e] + r * _ROUND_CAP:
                  min(starts[e] + (r + 1) * _ROUND_CAP, starts[e + 1])]
            for e in range(E)])
        if not len(sel):
            continue
        xr = xf[sel].reshape(1, len(sel), h)
        ir = idxf[sel].reshape(1, len(sel))
        out[sel] = _kernel_once(
            xr, ir, gate_proj, up_proj, down_proj).reshape(len(sel), h)
    return out.reshape(b, s, h)



# revision 2
# speedup vs baseline: 1.8146x; 1.8146x over previous
"""Expert-parallel MoE MLP kernel for Trainium2 (8 NeuronCores).

Problem: x[B=2,S=1024,H=1024] f32, expert_indices[B,S] int, 16 experts,
gate/up_proj[E,H,I], down_proj[E,I,H] (H=I=1024):
    out[n] = silu(x_n @ Wg[e_n]) * (x_n @ Wu[e_n]) @ Wd[e_n].T

Sharding: expert parallelism - core c owns experts {2c, 2c+1}. The host
groups tokens by expert (the "all-to-all dispatch" runs on host since the
kernel contract is full-input -> full-output), pads each expert's token
block to a 16-multiple capacity, and each core runs dense per-expert GEMMs.

All operands are bf16 (rel err ~4e-3 vs the 2e-2 gate), which halves the
mandatory weight traffic to 12 MB/core - the roofline. Perf structure:
  - weight DMAs ride the SP HWDGE queue alone, in stream order, sized
    0.5 MB (4 KB per-partition runs); nothing compute-dependent ever
    enters that FIFO, so the stream never stalls
  - xt and out DMAs ride the Activation HWDGE queue in parallel
  - matmuls run at the real (padded-to-16) token width: bf16 is
    1 cycle/row at any width, unlike f32r which needs >=256
  - gate/up accumulate h-outer into 8 PSUM banks so matmuls chase the
    arriving weight chunks; down_proj is packed in output-column slices
    so the last weight chunk feeds only 16 short matmuls + one 73 KB
    store (a ~1.5 us kernel tail)
"""

import math

import numpy as np

E = 16
H = 1024
HT = 8           # H / 128 partition tiles
N_CORES = 8
EPC = E // N_CORES   # experts per core
NS = 4           # weight DMA chunks per projection (0.5 MB each)
HH = HT // NS    # h-tiles per gate/up chunk
JT = H // NS // 128  # j-tiles per down chunk

_NC_CACHE = {}


def _build_nc(pio: int):
    """One SPMD program: EPC experts, pio real (DMA'd) token columns per
    expert. pio must be a multiple of 16, <= 512 (PSUM bank = 2 KB f32)."""
    import concourse.tile as tile
    from concourse import bacc, mybir
    from concourse.bass import ts

    f32 = mybir.dt.float32
    bf16 = mybir.dt.bfloat16

    nc = bacc.Bacc("TRN2", target_bir_lowering=False, debug=False,
                   num_devices=N_CORES)
    # gate/up packed partition-major: w[e, proj, p, h, :] = Wproj[e][h*128+p, :]
    w = nc.dram_tensor("w", [EPC, 2, 128, HT, H], bf16, kind="ExternalInput")
    # down packed j-sliced: wd[e, p, q, k, jj] = Wd[e].T[k*128+p, q*256+jj]
    wd = nc.dram_tensor("wd", [EPC, 128, NS, HT, H // NS], bf16,
                        kind="ExternalInput")
    xt = nc.dram_tensor("xt", [EPC, 128, HT, pio], bf16, kind="ExternalInput")
    out = nc.dram_tensor("out", [EPC, 128, HT, pio], bf16,
                         kind="ExternalOutput")

    with tile.TileContext(nc) as tc:
        with (
            tc.tile_pool(name="wp", bufs=6 * NS) as wp,
            tc.tile_pool(name="xp", bufs=1) as xp,
            tc.tile_pool(name="gp", bufs=2) as gp,
            tc.tile_pool(name="ip", bufs=2) as ip,
            tc.tile_pool(name="op", bufs=2) as op,
            tc.tile_pool(name="ps", bufs=8, space="PSUM") as ps,
        ):
            # tokens for both experts, on the Act queue (parallel to weights)
            x_sb = xp.tile([128, EPC, HT, pio], bf16)
            for e in range(EPC):
                nc.scalar.dma_start(x_sb[:, e], xt[e])

            # the whole weight stream, enqueued up front in use order; every
            # tile is a distinct buffer so the FIFO never waits on compute
            wts = {}
            for e in range(EPC):
                for proj in range(2):
                    for q in range(NS):
                        t = wp.tile([128, HH, H], bf16, tag="w",
                                    name=f"w{e}{proj}{q}")
                        nc.sync.dma_start(t[:], w[e, proj, :, ts(q, HH), :])
                        wts[e, proj, q] = t
                for q in range(NS):
                    t = wp.tile([128, HT, H // NS], bf16, tag="w",
                                name=f"wd{e}{q}")
                    nc.sync.dma_start(t[:], wd[e, :, q])
                    wts[e, 2, q] = t

            for e in range(EPC):
                g_sb = gp.tile([128, HT, pio], f32)
                i_sb = ip.tile([128, HT, pio], bf16)
                o_sb = op.tile([128, HT, pio], bf16, tag="o")
                # gate: h-outer accumulation into 8 PSUM banks
                g_ps = [ps.tile([128, pio], f32, tag="ps", name=f"g{e}{i}")
                        for i in range(HT)]
                for h in range(HT):
                    wt = wts[e, 0, h // HH]
                    for i in range(HT):
                        nc.tensor.matmul(
                            g_ps[i][:], wt[:, h % HH, ts(i, 128)],
                            x_sb[:, e, h], start=(h == 0), stop=(h == HT - 1))
                for i in range(HT):
                    nc.scalar.activation(
                        g_sb[:, i], g_ps[i][:],
                        mybir.ActivationFunctionType.Silu)
                # up
                u_ps = [ps.tile([128, pio], f32, tag="ps", name=f"u{e}{i}")
                        for i in range(HT)]
                for h in range(HT):
                    wt = wts[e, 1, h // HH]
                    for i in range(HT):
                        nc.tensor.matmul(
                            u_ps[i][:], wt[:, h % HH, ts(i, 128)],
                            x_sb[:, e, h], start=(h == 0), stop=(h == HT - 1))
                for i in range(HT):
                    nc.vector.tensor_mul(i_sb[:, i], g_sb[:, i], u_ps[i][:])
                # down: j-sliced chunks; each chunk finishes 2 j-tiles and
                # ships them immediately
                for q in range(NS):
                    wt = wts[e, 2, q]
                    for jl in range(JT):
                        o_ps = ps.tile([128, pio], f32, tag="ps",
                                       name=f"o{e}{q}{jl}")
                        for k in range(HT):
                            nc.tensor.matmul(
                                o_ps[:], wt[:, k, ts(jl, 128)], i_sb[:, k],
                                start=(k == 0), stop=(k == HT - 1))
                        nc.vector.tensor_copy(o_sb[:, q * JT + jl], o_ps[:])
                    nc.scalar.dma_start(out[e, :, ts(q, JT), :],
                                        o_sb[:, ts(q, JT), :])
    nc.compile()
    return nc


def _get_nc(pio: int):
    if pio not in _NC_CACHE:
        _NC_CACHE[pio] = _build_nc(pio)
    return _NC_CACHE[pio]


_ROUND_CAP = 512          # max tokens/expert per device round (PSUM bank)


def _kernel_once(x, expert_indices, gate_proj, up_proj, down_proj):
    import ml_dtypes
    from concourse.bass_utils import run_bass_kernel_spmd

    bf16 = np.dtype(ml_dtypes.bfloat16)
    x = np.ascontiguousarray(x, dtype=np.float32)
    b, s, h = x.shape
    assert (h, gate_proj.shape) == (H, (E, H, H)), (x.shape, gate_proj.shape)

    n = b * s
    xf = x.reshape(n, h)
    idx = np.asarray(expert_indices).reshape(n).astype(np.int64)

    order = np.argsort(idx, kind="stable")       # token ids grouped by expert
    counts = np.bincount(idx, minlength=E)
    starts = np.zeros(E + 1, dtype=np.int64)
    np.cumsum(counts, out=starts[1:])
    maxc = int(counts.max())
    assert maxc <= _ROUND_CAP
    pio = max(16, 16 * math.ceil(maxc / 16))

    # per-core weight packing (bf16, partition-major)
    wr = np.stack([gate_proj, up_proj], axis=1).astype(bf16) \
        .reshape(N_CORES, EPC, 2, HT, 128, H).transpose(0, 1, 2, 4, 3, 5)
    wdr = np.ascontiguousarray(down_proj.transpose(0, 2, 1)).astype(bf16) \
        .reshape(N_CORES, EPC, HT, 128, NS, H // NS).transpose(0, 1, 3, 4, 2, 5)
    in_maps = []
    tok_ids = []
    for c in range(N_CORES):
        xt_c = np.zeros((EPC, H, pio), dtype=np.float32)
        toks = []
        for le in range(EPC):
            e = c * EPC + le
            te = order[starts[e]:starts[e + 1]]
            toks.append(te)
            xt_c[le, :, :len(te)] = xf[te].T
        tok_ids.append(toks)
        in_maps.append({
            "w": np.ascontiguousarray(wr[c]),
            "wd": np.ascontiguousarray(wdr[c]),
            "xt": xt_c.astype(bf16).reshape(EPC, HT, 128, pio)
                  .transpose(0, 2, 1, 3).copy(),
        })

    nc = _get_nc(pio)
    res = run_bass_kernel_spmd(nc, in_maps, core_ids=list(range(N_CORES)))

    out = np.empty((n, h), dtype=np.float32)
    for c in range(N_CORES):
        o = res.results[c]["out"]                # [EPC, 128, HT, pio] bf16
        for le in range(EPC):
            te = tok_ids[c][le]
            oe = np.asarray(o[le]).astype(np.float32) \
                .transpose(1, 0, 2).reshape(h, pio)      # [H, pio]
            out[te] = oe[:, :len(te)].T
    return out.reshape(b, s, h)


def kernel(x, expert_indices, gate_proj, up_proj, down_proj):
    """Full-input -> full-output entry point.

    Tokens-per-expert above _ROUND_CAP (pathological skew; PSUM bound)
    are handled by running the device kernel in multiple rounds over
    disjoint token slices - outputs are per-token independent."""
    idx = np.asarray(expert_indices)
    counts = np.bincount(idx.reshape(-1).astype(np.int64), minlength=E)
    if counts.max() <= _ROUND_CAP:
        return _kernel_once(x, expert_indices, gate_proj, up_proj, down_proj)

    b, s, h = x.shape
    n = b * s
    xf = np.ascontiguousarray(x, dtype=np.float32).reshape(n, h)
    idxf = idx.reshape(n).astype(np.int64)
    order = np.argsort(idxf, kind="stable")
    starts = np.zeros(E + 1, dtype=np.int64)
    np.cumsum(np.bincount(idxf, minlength=E), out=starts[1:])
    out = np.empty((n, h), dtype=np.float32)
    rounds = math.ceil(counts.max() / _ROUND_CAP)
    for r in range(rounds):
        sel = np.concatenate([
            order[starts[e] + r * _ROUND_CAP:
                  min(starts[e] + (r + 1) * _ROUND_CAP, starts[e + 1])]
            for e in range(E)])
        if not len(sel):
            continue
        xr = xf[sel].reshape(1, len(sel), h)
        ir = idxf[sel].reshape(1, len(sel))
        out[sel] = _kernel_once(
            xr, ir, gate_proj, up_proj, down_proj).reshape(len(sel), h)
    return out.reshape(b, s, h)
